# revision 15
# baseline (speedup 1.0000x reference)
"""GNN message passing (gather + segment-sum) on 8 Trainium2 NeuronCores.

Strategy (node-range sharding per the spec's sharding_hint):
  - Destination nodes are range-sharded across the 8 cores (12500 nodes
    each), so each core owns a disjoint slice of the output and no
    cross-core reduction is needed.
  - The device-side gather uses the batched SWDGE row-gather
    (`nc.gpsimd.dma_gather`).  Its indices are int16, so x is packed as
    [25001, 256] (4 node rows per packed row plus one zero row) and
    source nodes are split into 4 "colors" by src % 4; color q gathers
    from column slice q*64:(q+1)*64 with elem_step=256 and index
    src//4 <= 25000.
  - Per core and color, the core's nodes are sorted by color-in-degree
    and grouped into 98 tiles of 128 (one SBUF partition per node).
    The slot schedule is K-MAJOR: round k touches the prefix of tiles
    whose max in-tile degree exceeds k, so consecutive slots are
    consecutive tiles and the accumulation is a handful of WIDE vector
    ops per gather call instead of one 64-wide op per slot.
  - Gather calls carry S=32 slots (4096 descriptors) to amortize the
    ~1us fixed SWDGE overhead; the descriptor ring is enlarged via
    dynamic_dma_scratch_size so a full call fits, and calls round-robin
    the 4 SWDGE queues so descriptor generation overlaps SDMA drain.
  - Accumulators stream back to DRAM per color; the host undoes the four
    degree-sort permutations, sums the color partials, and concatenates
    the 8 node-range slices.
"""

import numpy as np
from contextlib import ExitStack

import concourse.bacc as bacc
import concourse.bass as bass
import concourse.tile as tile
import concourse.mybir as mybir
from concourse.bass_utils import run_bass_kernel_spmd

N_NODES = 100000
N_EDGES = 1250000
D = 64
N_CORES = 8
NPC = N_NODES // N_CORES          # 12500 nodes per core
P = 128
TILES = (NPC + P - 1) // P        # 98 node tiles per core
NPC_PAD = TILES * P               # 12544
COLORS = 4
RPACK = N_NODES // COLORS + 1     # 25001 packed rows (last = zeros)
DUMMY = RPACK - 1
S = 8                             # gather slots per dma_gather call
                                  # (HW SWDGE ring: 1024 descriptors/queue)
SCRATCH = 16384                   # default SWDGE ring carveout
SINGLE_PACKET = True              # dma_gather packetization A/B knob

# Set by test.py for profiling; harness path leaves these untouched.
PROFILE = False
TRACE_CORES = None
LAST_EXEC_NS = None
LAST_RESULTS = None

_COMPILE_CACHE = {}


def _preprocess(edge_index, x):
    """Host-side sharding: per-core, per-color padded gather-index tables."""
    dest = np.asarray(edge_index[0]).astype(np.int64)
    src = np.asarray(edge_index[1]).astype(np.int64)
    x = np.ascontiguousarray(np.asarray(x), dtype=np.float32)

    x_pack = np.zeros((RPACK, COLORS * D), np.float32)
    x_pack[:N_NODES // COLORS] = x.reshape(N_NODES // COLORS, COLORS * D)

    core_of = dest // NPC
    # per (core, color): (perm, deg_pad, starts_pad, srcs_sorted)
    pc = [[None] * COLORS for _ in range(N_CORES)]
    K_all = np.zeros((N_CORES, COLORS, TILES), np.int64)
    for c in range(N_CORES):
        m = core_of == c
        d_loc = dest[m] - c * NPC
        s_c = src[m]
        color = s_c % COLORS
        for q in range(COLORS):
            mq = color == q
            d_q = d_loc[mq]
            s_q = (s_c[mq] // COLORS).astype(np.int16)
            deg = np.bincount(d_q, minlength=NPC)
            order = np.argsort(d_q, kind="stable")
            s_sorted = s_q[order]
            starts = np.zeros(NPC, np.int64)
            starts[1:] = np.cumsum(deg)[:-1]
            perm = np.argsort(-deg, kind="stable")
            deg_pad = np.concatenate([deg[perm],
                                      np.zeros(NPC_PAD - NPC, np.int64)])
            starts_pad = np.concatenate([starts[perm],
                                         np.zeros(NPC_PAD - NPC, np.int64)])
            K_all[c, q] = deg_pad.reshape(TILES, P)[:, 0]
            pc[c][q] = (perm, deg_pad, starts_pad, s_sorted)

    K = K_all.max(axis=0)                      # [COLORS, TILES] shared schedule

    # K-major slot sequence per color: round k covers the prefix of tiles
    # with K[q][t] > k; consecutive slots are consecutive tiles.
    slots = []                                 # (q, k, t) with t=-1 for pad
    calls = []                                 # (q, slot_lo) per call
    for q in range(COLORS):
        kmax = int(K[q][0])
        q_slots = []
        for k in range(kmax):
            n_k = int(np.count_nonzero(K[q] > k))
            q_slots.extend((q, k, t) for t in range(n_k))
        while len(q_slots) % S:
            q_slots.append((q, -1, -1))
        for i in range(0, len(q_slots), S):
            calls.append((q, len(slots) + i))
        slots.extend(q_slots)

    # Per-call vector-op runs: (k, t0, L, j0) — L consecutive tiles of
    # round k starting at call-local slot j0.
    call_runs = []
    for q, lo in calls:
        runs = []
        j = 0
        while j < S:
            qq, k, t = slots[lo + j]
            if t < 0:
                j += 1
                continue
            j0, t0 = j, t
            while (j + 1 < S
                   and slots[lo + j + 1] == (q, k, slots[lo + j][2] + 1)):
                j += 1
            runs.append((k, t0, j - j0 + 1, j0))
            j += 1
        call_runs.append(runs)

    # Per-core idx tables in slot order.
    n_slots = len(slots)
    n_calls = len(calls)
    idx_maps = []
    for c in range(N_CORES):
        vals = np.full((n_slots, P), DUMMY, np.int16)
        row = 0
        for q in range(COLORS):
            perm, deg_pad, starts_pad, s_sorted = pc[c][q]
            s_safe = np.concatenate([s_sorted, np.zeros(1, np.int16)])
            kmax = int(K[q][0])
            if kmax == 0:
                continue
            kk = np.arange(kmax)[None, :]
            dg = deg_pad[:, None]
            st = starts_pad[:, None]
            pos = np.minimum(st + kk, len(s_safe) - 1)
            V = np.where(kk < dg, s_safe[pos], DUMMY).astype(np.int16)
            Vt = V.reshape(TILES, P, kmax).transpose(2, 0, 1)  # [kmax,TILES,P]
            for k in range(kmax):
                n_k = int(np.count_nonzero(K[q] > k))
                vals[row:row + n_k] = Vt[k, :n_k]
                row += n_k
            row += (-row) % S                  # skip this color's pad rows
        # wrap: call block [S, P] -> ravel i=s*128+p -> [16, S*8] -> tile x8;
        # stored partition-major [128, n_calls, S*8] so one contiguous DMA
        # preloads every call's table.
        v = vals.reshape(n_calls, S * P)
        wrapped = v.reshape(n_calls, S * P // 16, 16).transpose(0, 2, 1)
        full = np.tile(wrapped, (1, 8, 1))     # [n_calls, 128, S*8]
        idx_maps.append(np.ascontiguousarray(full.transpose(1, 0, 2)))

    perms = [[pc[c][q][0] for q in range(COLORS)] for c in range(N_CORES)]
    K_key = tuple(tuple(int(v) for v in K[q]) for q in range(COLORS))
    return x_pack, idx_maps, perms, K_key, calls, call_runs


def _build_program(K, calls, call_runs):
    n_calls = len(calls)
    cols = S * P // 16
    nc = bacc.Bacc("TRN2", target_bir_lowering=False, debug=False,
                   num_devices=N_CORES, num_swdge_queues=4,
                   dynamic_dma_scratch_size=SCRATCH)
    x_dram = nc.dram_tensor("x", [RPACK, COLORS * D], mybir.dt.float32,
                            kind="ExternalInput")
    idx_dram = nc.dram_tensor("idx", [P, n_calls, cols], mybir.dt.int16,
                              kind="ExternalInput")
    out_dram = nc.dram_tensor("out", [COLORS, NPC_PAD, D], mybir.dt.float32,
                              kind="ExternalOutput")

    with tile.TileContext(nc) as tc, ExitStack() as ctx:
        idx_pool = ctx.enter_context(tc.tile_pool(name="idx", bufs=1))
        g_pool = ctx.enter_context(tc.tile_pool(name="g", bufs=16))
        acc_pool = ctx.enter_context(tc.tile_pool(name="acc", bufs=1))

        idx_all = idx_pool.tile([P, n_calls * cols], mybir.dt.int16,
                                tag="idx", name="idx_all")
        nc.sync.dma_start(
            out=idx_all[:],
            in_=idx_dram.ap().rearrange("p c j -> p (c j)"))
        acc = [acc_pool.tile([P, TILES * D], mybir.dt.float32,
                             tag=f"acc{q}", name=f"acc{q}")
               for q in range(COLORS)]
        for ci, (q, lo) in enumerate(calls):
            g = g_pool.tile([P, S, D], mybir.dt.float32, tag="g",
                            name=f"g{ci}")
            nc.gpsimd.dma_gather(
                out_ap=g[:],
                in_ap=x_dram.ap()[:, q * D:(q + 1) * D],
                idxs_ap=idx_all[:, ci * cols:(ci + 1) * cols],
                num_idxs=S * P,
                num_idxs_reg=S * P,
                elem_size=D,
                elem_step=COLORS * D,
                queue_num=ci % 4,
                single_packet=SINGLE_PACKET,
            )
            g2 = g[:].rearrange("p s d -> p (s d)")
            for k, t0, L, j0 in call_runs[ci]:
                src_ap = g2[:, j0 * D:(j0 + L) * D]
                dst_ap = acc[q][:, t0 * D:(t0 + L) * D]
                if k == 0:
                    nc.vector.tensor_copy(dst_ap, src_ap)
                else:
                    nc.vector.tensor_add(dst_ap, dst_ap, src_ap)
            # end of this color: zero never-written tiles, then store
            if ci + 1 == n_calls or calls[ci + 1][0] != q:
                for t in range(TILES):
                    if K[q][t] == 0:
                        nc.vector.memset(acc[q][:, bass.ts(t, D)], 0.0)
                nc.sync.dma_start(
                    out=out_dram.ap()[q].rearrange("(t p) d -> p t d", p=P),
                    in_=acc[q][:].rearrange("p (t d) -> p t d", d=D))
    nc.compile()
    return nc


def _install_profile_shim():
    """trace=True under axon needs the NTFF hook that this image's antenv
    lacks; register the ctypes-based one from trn_agent_boot."""
    import sys, types
    import concourse.bass_utils as bu
    if "antenv.axon_hooks" not in sys.modules:
        from trn_agent_boot.trn_boot import _ntff_profile_via_ctypes
        shim = types.ModuleType("antenv.axon_hooks")
        hook = _ntff_profile_via_ctypes("/opt/axon/libaxon_pjrt.so")
        shim.get_axon_ntff_profile_hook = lambda: hook
        shim.set_axon_ntff_profile_hook = lambda h: None
        sys.modules["antenv.axon_hooks"] = shim
    bu.upload_artifacts = lambda tmpdir: f"local:{tmpdir}"


def kernel(edge_index, x):
    global LAST_EXEC_NS, LAST_RESULTS
    x_pack, idx_maps, perms, K, calls, call_runs = _preprocess(edge_index, x)

    cache_key = (K, len(calls))
    if cache_key not in _COMPILE_CACHE:
        _COMPILE_CACHE[cache_key] = _build_program(K, calls, call_runs)
    nc = _COMPILE_CACHE[cache_key]

    in_maps = [{"x": x_pack, "idx": idx_maps[c]} for c in range(N_CORES)]
    kwargs = {}
    if PROFILE:
        _install_profile_shim()
        kwargs = dict(trace=True, trace_cores=TRACE_CORES)
    res = run_bass_kernel_spmd(nc, in_maps, core_ids=list(range(N_CORES)),
                               **kwargs)
    LAST_EXEC_NS = res.exec_time_ns
    LAST_RESULTS = res

    out = np.empty((N_NODES, D), np.float32)
    for c in range(N_CORES):
        dev = res.results[c]["out"]            # [COLORS, NPC_PAD, D]
        sl = np.zeros((NPC, D), np.float32)
        for q in range(COLORS):
            tmp = np.empty((NPC, D), np.float32)
            tmp[perms[c][q]] = dev[q][:NPC]
            sl += tmp
        out[c * NPC:(c + 1) * NPC] = sl
    return out


# revision 19
# speedup vs baseline: 1.1191x; 1.1191x over previous
"""GNN message passing (gather + segment-sum) on 8 Trainium2 NeuronCores.

Strategy (node-range sharding per the spec's sharding_hint):
  - Destination nodes are range-sharded across the 8 cores (12500 nodes
    each), so each core owns a disjoint slice of the output and no
    cross-core reduction is needed.
  - The device-side gather uses the batched SWDGE row-gather
    (`nc.gpsimd.dma_gather`).  Its indices are int16, so x is packed as
    [25001, 256] (4 node rows per packed row plus one zero row) and
    source nodes are split into 4 "colors" by src % 4; color q gathers
    from column slice q*64:(q+1)*64 with elem_step=256 and index
    src//4 <= 25000.
  - Per core and color, the core's nodes are sorted by color-in-degree
    and grouped into 98 tiles of 128 (one SBUF partition per node).
    The slot schedule is K-MAJOR: round k touches the prefix of tiles
    whose max in-tile degree exceeds k, so consecutive slots are
    consecutive tiles and the accumulation is a handful of WIDE vector
    ops per gather call instead of one 64-wide op per slot.
  - Gather calls carry S=32 slots (4096 descriptors) to amortize the
    ~1us fixed SWDGE overhead; the descriptor ring is enlarged via
    dynamic_dma_scratch_size so a full call fits, and calls round-robin
    the 4 SWDGE queues so descriptor generation overlaps SDMA drain.
  - Accumulators stream back to DRAM per color; the host undoes the four
    degree-sort permutations, sums the color partials, and concatenates
    the 8 node-range slices.
"""

import numpy as np
from contextlib import ExitStack

import concourse.bacc as bacc
import concourse.bass as bass
import concourse.tile as tile
import concourse.mybir as mybir
from concourse.bass_utils import run_bass_kernel_spmd

N_NODES = 100000
N_EDGES = 1250000
D = 64
N_CORES = 8
NPC = N_NODES // N_CORES          # 12500 nodes per core
P = 128
TILES = (NPC + P - 1) // P        # 98 node tiles per core
NPC_PAD = TILES * P               # 12544
COLORS = 4
RPACK = N_NODES // COLORS + 1     # 25001 packed rows (last = zeros)
DUMMY = RPACK - 1
S = 8                             # gather slots per dma_gather call
                                  # (HW SWDGE ring: 1024 descriptors/queue)
SCRATCH = 16384                   # default SWDGE ring carveout
SINGLE_PACKET = True              # dma_gather packetization A/B knob

# Set by test.py for profiling; harness path leaves these untouched.
PROFILE = False
TRACE_CORES = None
LAST_EXEC_NS = None
LAST_RESULTS = None

_COMPILE_CACHE = {}


def _preprocess(edge_index, x):
    """Host-side sharding: per-core, per-color padded gather-index tables."""
    dest = np.asarray(edge_index[0]).astype(np.int64)
    src = np.asarray(edge_index[1]).astype(np.int64)
    x = np.ascontiguousarray(np.asarray(x), dtype=np.float32)

    x_pack = np.zeros((RPACK, COLORS * D), np.float32)
    x_pack[:N_NODES // COLORS] = x.reshape(N_NODES // COLORS, COLORS * D)

    # Degree-balanced dest->core assignment: dests ranked by total in-degree
    # round-robin across cores, so every core sees the same degree profile
    # and the shared (max-over-cores) K schedule stays tight.
    total_deg = np.bincount(dest, minlength=N_NODES)
    rank = np.argsort(-total_deg, kind="stable")
    core_of_node = np.empty(N_NODES, np.int64)
    core_of_node[rank] = np.arange(N_NODES) % N_CORES
    dest_lists = [np.flatnonzero(core_of_node == c) for c in range(N_CORES)]
    local_id = np.empty(N_NODES, np.int64)
    for c in range(N_CORES):
        local_id[dest_lists[c]] = np.arange(len(dest_lists[c]))
    core_of = core_of_node[dest]
    # per (core, color): (perm, deg_pad, starts_pad, srcs_sorted)
    pc = [[None] * COLORS for _ in range(N_CORES)]
    K_all = np.zeros((N_CORES, COLORS, TILES), np.int64)
    for c in range(N_CORES):
        m = core_of == c
        d_loc = local_id[dest[m]]
        s_c = src[m]
        color = s_c % COLORS
        for q in range(COLORS):
            mq = color == q
            d_q = d_loc[mq]
            s_q = (s_c[mq] // COLORS).astype(np.int16)
            deg = np.bincount(d_q, minlength=NPC)
            order = np.argsort(d_q, kind="stable")
            s_sorted = s_q[order]
            starts = np.zeros(NPC, np.int64)
            starts[1:] = np.cumsum(deg)[:-1]
            perm = np.argsort(-deg, kind="stable")
            deg_pad = np.concatenate([deg[perm],
                                      np.zeros(NPC_PAD - NPC, np.int64)])
            starts_pad = np.concatenate([starts[perm],
                                         np.zeros(NPC_PAD - NPC, np.int64)])
            K_all[c, q] = deg_pad.reshape(TILES, P)[:, 0]
            pc[c][q] = (perm, deg_pad, starts_pad, s_sorted)

    K = K_all.max(axis=0)                      # [COLORS, TILES] shared schedule

    # K-major slot sequence per color: round k covers the prefix of tiles
    # with K[q][t] > k; consecutive slots are consecutive tiles.
    slots = []                                 # (q, k, t) with t=-1 for pad
    calls = []                                 # (q, slot_lo) per call
    for q in range(COLORS):
        kmax = int(K[q][0])
        q_slots = []
        for k in range(kmax):
            n_k = int(np.count_nonzero(K[q] > k))
            q_slots.extend((q, k, t) for t in range(n_k))
        while len(q_slots) % S:
            q_slots.append((q, -1, -1))
        for i in range(0, len(q_slots), S):
            calls.append((q, len(slots) + i))
        slots.extend(q_slots)

    # Per-call vector-op runs: (k, t0, L, j0) — L consecutive tiles of
    # round k starting at call-local slot j0.
    call_runs = []
    for q, lo in calls:
        runs = []
        j = 0
        while j < S:
            qq, k, t = slots[lo + j]
            if t < 0:
                j += 1
                continue
            j0, t0 = j, t
            while (j + 1 < S
                   and slots[lo + j + 1] == (q, k, slots[lo + j][2] + 1)):
                j += 1
            runs.append((k, t0, j - j0 + 1, j0))
            j += 1
        call_runs.append(runs)

    # Per-core idx tables in slot order.
    n_slots = len(slots)
    n_calls = len(calls)
    idx_maps = []
    for c in range(N_CORES):
        vals = np.full((n_slots, P), DUMMY, np.int16)
        row = 0
        for q in range(COLORS):
            perm, deg_pad, starts_pad, s_sorted = pc[c][q]
            s_safe = np.concatenate([s_sorted, np.zeros(1, np.int16)])
            kmax = int(K[q][0])
            if kmax == 0:
                continue
            kk = np.arange(kmax)[None, :]
            dg = deg_pad[:, None]
            st = starts_pad[:, None]
            pos = np.minimum(st + kk, len(s_safe) - 1)
            V = np.where(kk < dg, s_safe[pos], DUMMY).astype(np.int16)
            Vt = V.reshape(TILES, P, kmax).transpose(2, 0, 1)  # [kmax,TILES,P]
            for k in range(kmax):
                n_k = int(np.count_nonzero(K[q] > k))
                vals[row:row + n_k] = Vt[k, :n_k]
                row += n_k
            row += (-row) % S                  # skip this color's pad rows
        # wrap: call block [S, P] -> ravel i=s*128+p -> [16, S*8] -> tile x8;
        # stored partition-major [128, n_calls, S*8] so one contiguous DMA
        # preloads every call's table.
        v = vals.reshape(n_calls, S * P)
        wrapped = v.reshape(n_calls, S * P // 16, 16).transpose(0, 2, 1)
        full = np.tile(wrapped, (1, 8, 1))     # [n_calls, 128, S*8]
        idx_maps.append(np.ascontiguousarray(full.transpose(1, 0, 2)))

    perms = [[pc[c][q][0] for q in range(COLORS)] for c in range(N_CORES)]
    K_key = tuple(tuple(int(v) for v in K[q]) for q in range(COLORS))
    return x_pack, idx_maps, perms, K_key, calls, call_runs, dest_lists


def _build_program(K, calls, call_runs):
    n_calls = len(calls)
    cols = S * P // 16
    nc = bacc.Bacc("TRN2", target_bir_lowering=False, debug=False,
                   num_devices=N_CORES, num_swdge_queues=4,
                   dynamic_dma_scratch_size=SCRATCH)
    x_dram = nc.dram_tensor("x", [RPACK, COLORS * D], mybir.dt.float32,
                            kind="ExternalInput")
    idx_dram = nc.dram_tensor("idx", [P, n_calls, cols], mybir.dt.int16,
                              kind="ExternalInput")
    out_dram = nc.dram_tensor("out", [COLORS, NPC_PAD, D], mybir.dt.float32,
                              kind="ExternalOutput")

    with tile.TileContext(nc) as tc, ExitStack() as ctx:
        idx_pool = ctx.enter_context(tc.tile_pool(name="idx", bufs=1))
        g_pool = ctx.enter_context(tc.tile_pool(name="g", bufs=16))
        acc_pool = ctx.enter_context(tc.tile_pool(name="acc", bufs=1))

        # Preload idx tables per color so the first gather only waits on the
        # first chunk while the rest stream in behind it.
        idx_all = idx_pool.tile([P, n_calls * cols], mybir.dt.int16,
                                tag="idx", name="idx_all")
        qlo = [min((i for i, (qq, _) in enumerate(calls) if qq == q),
                   default=n_calls) for q in range(COLORS)] + [n_calls]
        for q in range(COLORS):
            a, b = qlo[q], qlo[q + 1]
            if a == b:
                continue
            nc.sync.dma_start(
                out=idx_all[:, a * cols:b * cols],
                in_=idx_dram.ap()[:, a:b].rearrange("p c j -> p (c j)"))
        acc = [acc_pool.tile([P, TILES * D], mybir.dt.float32,
                             tag=f"acc{q}", name=f"acc{q}")
               for q in range(COLORS)]
        for ci, (q, lo) in enumerate(calls):
            g = g_pool.tile([P, S, D], mybir.dt.float32, tag="g",
                            name=f"g{ci}")
            nc.gpsimd.dma_gather(
                out_ap=g[:],
                in_ap=x_dram.ap()[:, q * D:(q + 1) * D],
                idxs_ap=idx_all[:, ci * cols:(ci + 1) * cols],
                num_idxs=S * P,
                num_idxs_reg=S * P,
                elem_size=D,
                elem_step=COLORS * D,
                queue_num=ci % 4,
                single_packet=SINGLE_PACKET,
            )
            g2 = g[:].rearrange("p s d -> p (s d)")
            for k, t0, L, j0 in call_runs[ci]:
                src_ap = g2[:, j0 * D:(j0 + L) * D]
                dst_ap = acc[q][:, t0 * D:(t0 + L) * D]
                if k == 0:
                    nc.vector.tensor_copy(dst_ap, src_ap)
                else:
                    nc.vector.tensor_add(dst_ap, dst_ap, src_ap)
            # end of this color: zero never-written tiles, then store
            if ci + 1 == n_calls or calls[ci + 1][0] != q:
                for t in range(TILES):
                    if K[q][t] == 0:
                        nc.vector.memset(acc[q][:, bass.ts(t, D)], 0.0)
                nc.sync.dma_start(
                    out=out_dram.ap()[q].rearrange("(t p) d -> p t d", p=P),
                    in_=acc[q][:].rearrange("p (t d) -> p t d", d=D))
    nc.compile()
    return nc


def _install_profile_shim():
    """trace=True under axon needs the NTFF hook that this image's antenv
    lacks; register the ctypes-based one from trn_agent_boot."""
    import sys, types
    import concourse.bass_utils as bu
    if "antenv.axon_hooks" not in sys.modules:
        from trn_agent_boot.trn_boot import _ntff_profile_via_ctypes
        shim = types.ModuleType("antenv.axon_hooks")
        hook = _ntff_profile_via_ctypes("/opt/axon/libaxon_pjrt.so")
        shim.get_axon_ntff_profile_hook = lambda: hook
        shim.set_axon_ntff_profile_hook = lambda h: None
        sys.modules["antenv.axon_hooks"] = shim
    bu.upload_artifacts = lambda tmpdir: f"local:{tmpdir}"


def kernel(edge_index, x):
    global LAST_EXEC_NS, LAST_RESULTS
    (x_pack, idx_maps, perms, K, calls, call_runs,
     dest_lists) = _preprocess(edge_index, x)

    cache_key = (K, len(calls))
    if cache_key not in _COMPILE_CACHE:
        _COMPILE_CACHE[cache_key] = _build_program(K, calls, call_runs)
    nc = _COMPILE_CACHE[cache_key]

    in_maps = [{"x": x_pack, "idx": idx_maps[c]} for c in range(N_CORES)]
    kwargs = {}
    if PROFILE:
        _install_profile_shim()
        kwargs = dict(trace=True, trace_cores=TRACE_CORES)
    res = run_bass_kernel_spmd(nc, in_maps, core_ids=list(range(N_CORES)),
                               **kwargs)
    LAST_EXEC_NS = res.exec_time_ns
    LAST_RESULTS = res

    out = np.empty((N_NODES, D), np.float32)
    for c in range(N_CORES):
        dev = res.results[c]["out"]            # [COLORS, NPC_PAD, D]
        sl = np.zeros((NPC, D), np.float32)
        for q in range(COLORS):
            tmp = np.empty((NPC, D), np.float32)
            tmp[perms[c][q]] = dev[q][:NPC]
            sl += tmp
        out[dest_lists[c]] = sl
    return out
